# revision 3
# baseline (speedup 1.0000x reference)
"""Trainium2 Bass kernel for blended-expert 3-layer MLP (moe_routing).

Math (per sample b):
  h1 = elu(sum_e blend[e,b] * (W1[e] @ x[b]  + b1[e]))
  h2 = elu(sum_e blend[e,b] * (W2[e] @ h1[b] + b2[e]))
  y  = softmax(sum_e blend[e,b] * (W3[e] @ h2[b] + b3[e]))

Strategy (per core, data-parallel over batch: B=8192 -> Bc=1024 per core):
  - Activations live TRANSPOSED in SBUF: hT[d, b] (d on partitions). Host
    pre-transposes x; host un-transposes the [363, Bc] output.
  - Blended linear as one PSUM accumulation: for each expert e the moving
    operand is rhs_e = hT * blend[e, :] (DVE tensor_tensor with a
    host-replicated broadcast tile), the stationary is a chunk of W_e^T.
    The blended bias enters the same PSUM group via a K=8 matmul
    (stationary = bias matrix [8, out], moving = blend [8, b]).
  - fp32r matmuls (11-bit mantissa, 1 cycle/row at N=512).
  - ELU drain: ACT exp + DVE tensor_scalar/scalar_tensor_tensor:
      elu(v) = max(v, 0) + min(exp(v) - 1, 0)
  - Softmax over the output dim (on partitions) without max-subtraction:
    exp via ACT, partition sums via a ones-stationary matmul, reciprocal on
    DVE, partition-broadcast on GPSIMD, final scale on DVE.
"""

import numpy as np

import concourse.bass as bass
import concourse.mybir as mybir
import concourse.tile as tile
from concourse import bacc
from concourse.bass_utils import run_bass_kernel_spmd

F32 = mybir.dt.float32
F32R = mybir.dt.float32r
AF = mybir.ActivationFunctionType
OP = mybir.AluOpType

N_CORES = 8
E = 8
B = 8192
BC = B // N_CORES          # 1024 per core
BT = 2                     # batch halves per core (PSUM free dim = 512)
BW = BC // BT              # 512
D0, D1, D2, D3 = 480, 512, 512, 363
D0P = 512                  # input dim padded to 4 K-chunks of 128
KC = 4                     # K chunks of 128 per expert (all layers)
# layer table: (out_dim, n_otiles, bias column offset)
LAYERS = [(D1, 4, 0), (D2, 4, D1), (D3, 3, D1 + D2)]
BIAS_W = D1 + D2 + D3


def _round_f32r(a):
    """Round-to-nearest-even fp32 -> fp32r (11-bit mantissa, low 12 bits 0)."""
    b = np.ascontiguousarray(a, dtype=np.float32).view(np.uint32)
    lo = b & np.uint32(0xFFF)
    hi = b >> np.uint32(12)
    round_up = (lo > 0x800) | ((lo == 0x800) & ((hi & 1) == 1))
    hi = hi + round_up.astype(np.uint32)
    return (hi << np.uint32(12)).view(np.float32)


def _build_program():
    nc = bacc.Bacc("TRN2", target_bir_lowering=False, debug=False,
                   num_devices=N_CORES)

    xt_d = nc.dram_tensor("xt", [128, KC * BC], F32, kind="ExternalInput").ap()
    bc_d = nc.dram_tensor("bcast", [128, E * BC], F32, kind="ExternalInput").ap()
    bl_d = nc.dram_tensor("blend", [E, BC], F32R, kind="ExternalInput").ap()
    bias_d = nc.dram_tensor("bias", [E, BIAS_W], F32R, kind="ExternalInput").ap()
    ones_d = nc.dram_tensor("ones", [128, 1], F32R, kind="ExternalInput").ap()
    w_d = [
        nc.dram_tensor("w1", [128, E * KC * D1], F32R, kind="ExternalInput").ap(),
        nc.dram_tensor("w2", [128, E * KC * D2], F32R, kind="ExternalInput").ap(),
        nc.dram_tensor("w3", [128, E * KC * D3], F32R, kind="ExternalInput").ap(),
    ]
    y_d = nc.dram_tensor("y", [D3, BC], F32, kind="ExternalOutput").ap()

    with tile.TileContext(nc) as tc:
        with (
            tc.tile_pool(name="const", bufs=1) as cpool,
            tc.tile_pool(name="acts", bufs=1) as apool,
            tc.tile_pool(name="wchunk", bufs=3) as wpool,
            tc.tile_pool(name="rhs", bufs=6) as rpool,
            tc.tile_pool(name="drain", bufs=6) as dpool,
            tc.tile_pool(name="psum", bufs=8, space="PSUM") as ppool,
        ):
            xt = cpool.tile([128, KC, BC], F32)
            bcast = cpool.tile([128, E, BC], F32)
            blend = cpool.tile([E, BC], F32R)
            bias = cpool.tile([E, BIAS_W], F32R)
            ones = cpool.tile([128, 1], F32R)
            nc.sync.dma_start(out=xt[:], in_=xt_d[:])
            nc.sync.dma_start(out=bcast[:], in_=bc_d[:])
            nc.sync.dma_start(out=blend[:], in_=bl_d[:])
            nc.sync.dma_start(out=bias[:], in_=bias_d[:])
            nc.sync.dma_start(out=ones[:], in_=ones_d[:])

            h1 = apool.tile([128, KC, BC], F32)
            h2 = apool.tile([128, KC, BC], F32)
            srcs = [xt, h1, h2]

            for li, (dout, n_ot, boff) in enumerate(LAYERS):
                src = srcs[li]
                # psum accumulators: one bank per (bt, ot)
                ps = [[ppool.tile([128, 512], F32, tag="psum",
                                  name=f"ps_l{li}_b{bt}_o{ot}")
                       for ot in range(n_ot)]
                      for bt in range(BT)]
                # blended bias seeds the accumulation
                for bt in range(BT):
                    bsl = bass.ts(bt, BW)
                    for ot in range(n_ot):
                        otw = min(128, dout - ot * 128)
                        nc.tensor.matmul(
                            ps[bt][ot][0:otw, :],
                            bias[:, boff + ot * 128: boff + ot * 128 + otw],
                            blend[:, bsl],
                            start=True, stop=False,
                        )
                for e in range(E):
                    w = wpool.tile([128, KC * 512], F32R, tag="w")
                    nc.sync.dma_start(
                        out=w[:, 0:KC * dout],
                        in_=w_d[li][:, e * KC * dout:(e + 1) * KC * dout],
                    )
                    for kc in range(KC):
                        for bt in range(BT):
                            bsl = bass.ts(bt, BW)
                            rhs = rpool.tile([128, BW], F32R)
                            nc.vector.tensor_tensor(
                                rhs[:], src[:, kc, bsl], bcast[:, e, bsl],
                                OP.mult,
                            )
                            last = (e == E - 1) and (kc == KC - 1)
                            for ot in range(n_ot):
                                otw = min(128, dout - ot * 128)
                                wsl = w[:, kc * dout + ot * 128:
                                        kc * dout + ot * 128 + otw]
                                nc.tensor.matmul(
                                    ps[bt][ot][0:otw, :], wsl, rhs[:],
                                    start=False, stop=last,
                                )
                if li < 2:
                    # ELU drain into the next layer's transposed activations
                    hnext = srcs[li + 1]
                    for bt in range(BT):
                        bsl = bass.ts(bt, BW)
                        for ot in range(n_ot):
                            p = ps[bt][ot]
                            et = dpool.tile([128, BW], F32, tag="et")
                            nc.scalar.activation(et[:], p[:], AF.Exp)
                            # et = min(exp(v) - 1, 0)   (2x-mode tensor_scalar)
                            nc.vector.tensor_scalar(
                                et[:], et[:], 1.0, 0.0, OP.subtract, OP.min)
                            # h = max(v, 0) + et
                            nc.vector.scalar_tensor_tensor(
                                hnext[:, ot, bsl], p[:], 0.0, et[:],
                                OP.max, OP.add)
                else:
                    # softmax over the partition (output) dim
                    for bt in range(BT):
                        bsl = bass.ts(bt, BW)
                        exs = []
                        sm = ppool.tile([128, 512], F32, tag="psum")
                        for ot in range(n_ot):
                            otw = min(128, dout - ot * 128)
                            p = ps[bt][ot]
                            ex = dpool.tile([128, BW], F32, tag="et")
                            exs.append((ex, otw))
                            nc.scalar.activation(
                                ex[0:otw, :].bitcast(F32R), p[0:otw, :], AF.Exp)
                            nc.tensor.matmul(
                                sm[0:1, :], ones[0:otw, 0:1],
                                ex[0:otw, :].bitcast(F32R),
                                start=(ot == 0), stop=(ot == n_ot - 1),
                            )
                        recip = dpool.tile([1, BW], F32, tag="recip")
                        nc.vector.reciprocal(recip[:], sm[0:1, :])
                        recipb = dpool.tile([128, BW], F32, tag="recipb")
                        nc.gpsimd.partition_broadcast(recipb[:], recip[:])
                        for ot, (ex, otw) in enumerate(exs):
                            yt = dpool.tile([128, BW], F32, tag="yt",
                                            name=f"yt_b{bt}_o{ot}")
                            nc.vector.tensor_tensor(
                                yt[0:otw, :], ex[0:otw, :], recipb[0:otw, :],
                                OP.mult)
                            nc.sync.dma_start(
                                out=y_d[ot * 128: ot * 128 + otw, bsl],
                                in_=yt[0:otw, :])
    nc.compile()
    return nc


_NC_CACHE = None


def _get_program():
    global _NC_CACHE
    if _NC_CACHE is None:
        _NC_CACHE = _build_program()
    return _NC_CACHE


def _prep_inputs(x, weight_blend, W1, b1, W2, b2, W3, b3):
    x = np.asarray(x, np.float32)
    blend = np.asarray(weight_blend, np.float32)

    xp = np.zeros((B, D0P), np.float32)
    xp[:, :D0] = x
    xT = np.ascontiguousarray(xp.T)                      # [512, B]

    def pack_w(W, din):
        # W: (E, dout, din) -> [128, E*KC*dout], chunk (e,kc) at col (e*KC+kc)*dout
        Wt = np.zeros((E, KC * 128, W.shape[1]), np.float32)
        Wt[:, :din, :] = np.transpose(W, (0, 2, 1))
        # (E, KC, 128, dout) -> (128, E, KC, dout)
        return _round_f32r(
            np.ascontiguousarray(
                Wt.reshape(E, KC, 128, W.shape[1])
                .transpose(2, 0, 1, 3)
                .reshape(128, -1)))

    w1h = pack_w(np.asarray(W1, np.float32), D0)
    w2h = pack_w(np.asarray(W2, np.float32), D1)
    w3h = pack_w(np.asarray(W3, np.float32), D2)
    bias_h = _round_f32r(np.concatenate(
        [np.asarray(b1, np.float32), np.asarray(b2, np.float32),
         np.asarray(b3, np.float32)], axis=1))
    ones_h = np.ones((128, 1), np.float32)

    in_maps = []
    for c in range(N_CORES):
        csl = slice(c * BC, (c + 1) * BC)
        xt_c = np.ascontiguousarray(
            xT[:, csl].reshape(KC, 128, BC).transpose(1, 0, 2).reshape(128, -1))
        bl_c = np.ascontiguousarray(blend[:, csl])
        bc_c = np.ascontiguousarray(
            np.broadcast_to(bl_c[None, :, :], (128, E, BC)).reshape(128, -1))
        in_maps.append({
            "xt": xt_c,
            "bcast": bc_c,
            "blend": _round_f32r(bl_c),
            "bias": bias_h,
            "ones": ones_h,
            "w1": w1h, "w2": w2h, "w3": w3h,
        })
    return in_maps


def run(inputs, trace=False, trace_kwargs=None):
    nc = _get_program()
    in_maps = _prep_inputs(
        inputs["x"], inputs["weight_blend"],
        inputs["W1"], inputs["b1"], inputs["W2"], inputs["b2"],
        inputs["W3"], inputs["b3"])
    res = run_bass_kernel_spmd(
        nc, in_maps, list(range(N_CORES)),
        trace=trace, **(trace_kwargs or {}))
    y = np.concatenate([res.results[c]["y"] for c in range(N_CORES)], axis=1)
    return np.ascontiguousarray(y.T), res


def kernel(**inputs):
    y, _ = run(inputs, trace=False)
    return y


# revision 7
# speedup vs baseline: 91.2134x; 91.2134x over previous
"""Trainium2 Bass kernel for blended-expert 3-layer MLP (moe_routing).

Math (per sample b):
  h1 = elu(sum_e blend[e,b] * (W1[e] @ x[b]  + b1[e]))
  h2 = elu(sum_e blend[e,b] * (W2[e] @ h1[b] + b2[e]))
  y  = softmax(sum_e blend[e,b] * (W3[e] @ h2[b] + b3[e]))

Strategy (per core, data-parallel over batch: B=8192 -> Bc=1024 per core):
  - Activations live TRANSPOSED in SBUF: hT[d, b] (d on partitions). Host
    pre-transposes x; host un-transposes the [363, Bc] output.
  - Blended linear as one PSUM accumulation: for each expert e the moving
    operand is rhs_e = hT * blend[e, :] (DVE tensor_tensor with a
    host-replicated broadcast tile), the stationary is a chunk of W_e^T.
    The blended bias enters the same PSUM group via a K=8 matmul
    (stationary = bias matrix [8, out], moving = blend [8, b]).
  - fp32r matmuls (11-bit mantissa, 1 cycle/row at N=512).
  - ELU drain: ACT exp + DVE tensor_scalar/scalar_tensor_tensor:
      elu(v) = max(v, 0) + min(exp(v) - 1, 0)
  - Softmax over the output dim (on partitions) without max-subtraction:
    exp via ACT, partition sums via a ones-stationary matmul, reciprocal on
    DVE, partition-broadcast on GPSIMD, final scale on DVE.
"""

import numpy as np

import concourse.bass as bass
import concourse.mybir as mybir
import concourse.tile as tile
from concourse import bacc
from concourse.bass_utils import run_bass_kernel_spmd

F32 = mybir.dt.float32
F32R = mybir.dt.float32r
AF = mybir.ActivationFunctionType
OP = mybir.AluOpType

N_CORES = 8
E = 8
B = 8192
BC = B // N_CORES          # 1024 per core
BT = 2                     # batch halves per core (PSUM free dim = 512)
BW = BC // BT              # 512
D0, D1, D2, D3 = 480, 512, 512, 363
D0P = 512                  # input dim padded to 4 K-chunks of 128
KC = 4                     # K chunks of 128 per expert (all layers)
# layer table: (out_dim, n_otiles, bias column offset)
LAYERS = [(D1, 4, 0), (D2, 4, D1), (D3, 3, D1 + D2)]
BIAS_W = D1 + D2 + D3


def _round_f32r(a):
    """Round-to-nearest-even fp32 -> fp32r (11-bit mantissa, low 12 bits 0)."""
    b = np.ascontiguousarray(a, dtype=np.float32).view(np.uint32)
    lo = b & np.uint32(0xFFF)
    hi = b >> np.uint32(12)
    round_up = (lo > 0x800) | ((lo == 0x800) & ((hi & 1) == 1))
    hi = hi + round_up.astype(np.uint32)
    return (hi << np.uint32(12)).view(np.float32)


def _build_program(reps=1):
    nc = bacc.Bacc("TRN2", target_bir_lowering=False, debug=False,
                   num_devices=N_CORES)

    xt_d = nc.dram_tensor("xt", [128, KC * BC], F32, kind="ExternalInput").ap()
    bc_d = nc.dram_tensor("bcast", [128, E * BC], F32, kind="ExternalInput").ap()
    bl_d = nc.dram_tensor("blend", [E, BC], F32R, kind="ExternalInput").ap()
    bias_d = nc.dram_tensor("bias", [E, BIAS_W], F32R, kind="ExternalInput").ap()
    ones_d = nc.dram_tensor("ones", [128, 1], F32R, kind="ExternalInput").ap()
    w_d = [
        nc.dram_tensor("w1", [128, E * KC * D1], F32R, kind="ExternalInput").ap(),
        nc.dram_tensor("w2", [128, E * KC * D2], F32R, kind="ExternalInput").ap(),
        nc.dram_tensor("w3", [128, E * KC * D3], F32R, kind="ExternalInput").ap(),
    ]
    y_d = nc.dram_tensor("y", [D3, BC], F32, kind="ExternalOutput").ap()

    with tile.TileContext(nc) as tc:
        with (
            tc.tile_pool(name="const", bufs=1) as cpool,
            tc.tile_pool(name="acts", bufs=1) as apool,
            tc.tile_pool(name="wchunk", bufs=3) as wpool,
            tc.tile_pool(name="rhs", bufs=6) as rpool,
            tc.tile_pool(name="drain", bufs=6) as dpool,
            tc.tile_pool(name="psum", bufs=8, space="PSUM") as ppool,
        ):
            xt = cpool.tile([128, KC, BC], F32)
            bcast = cpool.tile([128, E, BC], F32)
            blend = cpool.tile([E, BC], F32R)
            bias = cpool.tile([E, BIAS_W], F32R)
            ones = cpool.tile([128, 1], F32R)
            nc.sync.dma_start(out=xt[:], in_=xt_d[:])
            nc.sync.dma_start(out=bcast[:], in_=bc_d[:])
            nc.sync.dma_start(out=blend[:], in_=bl_d[:])
            nc.sync.dma_start(out=bias[:], in_=bias_d[:])
            nc.sync.dma_start(out=ones[:], in_=ones_d[:])

            h1 = apool.tile([128, KC, BC], F32)
            h2 = apool.tile([128, KC, BC], F32)
            srcs = [xt, h1, h2]

            def body():
                _network(nc, tc, srcs, bcast, blend, bias, ones,
                         w_d, y_d, wpool, rpool, dpool, ppool)

            if reps == 1:
                body()
            else:
                with tc.For_i(0, reps, 1):
                    body()
    nc.compile()
    return nc


def _network(nc, tc, srcs, bcast, blend, bias, ones, w_d, y_d,
             wpool, rpool, dpool, ppool):
    if True:
        if True:
            for li, (dout, n_ot, boff) in enumerate(LAYERS):
                src = srcs[li]
                # psum accumulators: one bank per (bt, ot)
                ps = [[ppool.tile([128, 512], F32, tag="psum",
                                  name=f"ps_l{li}_b{bt}_o{ot}")
                       for ot in range(n_ot)]
                      for bt in range(BT)]
                # blended bias seeds the accumulation
                for bt in range(BT):
                    bsl = bass.ts(bt, BW)
                    for ot in range(n_ot):
                        otw = min(128, dout - ot * 128)
                        nc.tensor.matmul(
                            ps[bt][ot][0:otw, :],
                            bias[:, boff + ot * 128: boff + ot * 128 + otw],
                            blend[:, bsl],
                            start=True, stop=False,
                        )
                for e in range(E):
                    w = wpool.tile([128, KC * 512], F32R, tag="w")
                    nc.sync.dma_start(
                        out=w[:, 0:KC * dout],
                        in_=w_d[li][:, e * KC * dout:(e + 1) * KC * dout],
                    )
                    for kc in range(KC):
                        for bt in range(BT):
                            bsl = bass.ts(bt, BW)
                            rhs = rpool.tile([128, BW], F32R)
                            nc.vector.tensor_tensor(
                                rhs[:], src[:, kc, bsl], bcast[:, e, bsl],
                                OP.mult,
                            )
                            last = (e == E - 1) and (kc == KC - 1)
                            for ot in range(n_ot):
                                otw = min(128, dout - ot * 128)
                                wsl = w[:, kc * dout + ot * 128:
                                        kc * dout + ot * 128 + otw]
                                nc.tensor.matmul(
                                    ps[bt][ot][0:otw, :], wsl, rhs[:],
                                    start=False, stop=last,
                                )
                if li < 2:
                    # ELU drain into the next layer's transposed activations
                    hnext = srcs[li + 1]
                    for bt in range(BT):
                        bsl = bass.ts(bt, BW)
                        for ot in range(n_ot):
                            p = ps[bt][ot]
                            et = dpool.tile([128, BW], F32, tag="et")
                            nc.scalar.activation(et[:], p[:], AF.Exp)
                            # et = min(exp(v) - 1, 0)   (2x-mode tensor_scalar)
                            nc.vector.tensor_scalar(
                                et[:], et[:], 1.0, 0.0, OP.subtract, OP.min)
                            # h = max(v, 0) + et
                            nc.vector.scalar_tensor_tensor(
                                hnext[:, ot, bsl], p[:], 0.0, et[:],
                                OP.max, OP.add)
                else:
                    # softmax over the partition (output) dim
                    for bt in range(BT):
                        bsl = bass.ts(bt, BW)
                        exs = []
                        sm = ppool.tile([128, 512], F32, tag="psum")
                        for ot in range(n_ot):
                            otw = min(128, dout - ot * 128)
                            p = ps[bt][ot]
                            ex = dpool.tile([128, BW], F32, tag="et")
                            exs.append((ex, otw))
                            nc.scalar.activation(
                                ex[0:otw, :].bitcast(F32R), p[0:otw, :], AF.Exp)
                            nc.tensor.matmul(
                                sm[0:1, :], ones[0:otw, 0:1],
                                ex[0:otw, :].bitcast(F32R),
                                start=(ot == 0), stop=(ot == n_ot - 1),
                            )
                        recip = dpool.tile([1, BW], F32, tag="recip")
                        nc.vector.reciprocal(recip[:], sm[0:1, :])
                        recipb = dpool.tile([128, BW], F32, tag="recipb")
                        nc.gpsimd.partition_broadcast(recipb[:], recip[:])
                        for ot, (ex, otw) in enumerate(exs):
                            yt = dpool.tile([128, BW], F32, tag="yt",
                                            name=f"yt_b{bt}_o{ot}")
                            nc.vector.tensor_tensor(
                                yt[0:otw, :], ex[0:otw, :], recipb[0:otw, :],
                                OP.mult)
                            nc.sync.dma_start(
                                out=y_d[ot * 128: ot * 128 + otw, bsl],
                                in_=yt[0:otw, :])


_NC_CACHE = {}


def _get_program(reps=1):
    if reps not in _NC_CACHE:
        _NC_CACHE[reps] = _build_program(reps)
    return _NC_CACHE[reps]


def _prep_inputs(x, weight_blend, W1, b1, W2, b2, W3, b3):
    x = np.asarray(x, np.float32)
    blend = np.asarray(weight_blend, np.float32)

    xp = np.zeros((B, D0P), np.float32)
    xp[:, :D0] = x
    xT = np.ascontiguousarray(xp.T)                      # [512, B]

    def pack_w(W, din):
        # W: (E, dout, din) -> [128, E*KC*dout], chunk (e,kc) at col (e*KC+kc)*dout
        Wt = np.zeros((E, KC * 128, W.shape[1]), np.float32)
        Wt[:, :din, :] = np.transpose(W, (0, 2, 1))
        # (E, KC, 128, dout) -> (128, E, KC, dout)
        return _round_f32r(
            np.ascontiguousarray(
                Wt.reshape(E, KC, 128, W.shape[1])
                .transpose(2, 0, 1, 3)
                .reshape(128, -1)))

    w1h = pack_w(np.asarray(W1, np.float32), D0)
    w2h = pack_w(np.asarray(W2, np.float32), D1)
    w3h = pack_w(np.asarray(W3, np.float32), D2)
    bias_h = _round_f32r(np.concatenate(
        [np.asarray(b1, np.float32), np.asarray(b2, np.float32),
         np.asarray(b3, np.float32)], axis=1))
    ones_h = np.ones((128, 1), np.float32)

    in_maps = []
    for c in range(N_CORES):
        csl = slice(c * BC, (c + 1) * BC)
        xt_c = np.ascontiguousarray(
            xT[:, csl].reshape(KC, 128, BC).transpose(1, 0, 2).reshape(128, -1))
        bl_c = np.ascontiguousarray(blend[:, csl])
        bc_c = np.ascontiguousarray(
            np.broadcast_to(bl_c[None, :, :], (128, E, BC)).reshape(128, -1))
        in_maps.append({
            "xt": xt_c,
            "bcast": bc_c,
            "blend": _round_f32r(bl_c),
            "bias": bias_h,
            "ones": ones_h,
            "w1": w1h, "w2": w2h, "w3": w3h,
        })
    return in_maps


def run(inputs, trace=False, trace_kwargs=None, reps=1):
    nc = _get_program(reps)
    in_maps = _prep_inputs(
        inputs["x"], inputs["weight_blend"],
        inputs["W1"], inputs["b1"], inputs["W2"], inputs["b2"],
        inputs["W3"], inputs["b3"])
    res = run_bass_kernel_spmd(
        nc, in_maps, list(range(N_CORES)),
        trace=trace, **(trace_kwargs or {}))
    y = np.concatenate([res.results[c]["y"] for c in range(N_CORES)], axis=1)
    return np.ascontiguousarray(y.T), res


def kernel(**inputs):
    y, _ = run(inputs, trace=False)
    return y
